# revision 24
# baseline (speedup 1.0000x reference)
"""Multi-Head Latent Attention (MLA) TRN2 Bass kernel.

Sharding: data-parallel over batch (B=2) x tensor-parallel over heads
(16 heads -> 4 per core) = 8 cores. The kv_lora latent path and shared
rope key are computed replicated within each batch group; the final
output projection is computed as per-core partials which the host sums.

All on-device dataflow is "transposed" (feature dim on partitions,
sequence on the free dim) so no PE transposes are ever needed:
  qT      = Wq_perm^T @ xT          [768, S]   via fp8-e4m3 DoubleRow
                                               matmuls (K=256/step, 2x rate)
  kv_aT   = Wkv_a^T @ xT            [576, S]   bf16 (c_kvT rows 0..511,
                                               k_ropeT rows 512..575)
  k_nopeT = Wkv_b_k^T @ c_kvT       [512, S]
  v       = (c_kvT chunk)^T-matmuls [S, 512]   (natural layout)
  RoPE applied in transposed layout with a partition-swap DMA + 3 DVE ops
  scoresT[s_k, s_q] per (head, q-block of 512), causal masks added on the
  4 diagonal chunks, exp on ACT (no max subtraction; scores are bounded),
  softmax denominators via DVE accumulation of the exp tiles + a single
  ones-matmul per (head, q-block), out^T accumulated in PSUM, normalized
  by broadcasted reciprocals, then partialT = Wo_c^T @ outT in bf16.

The q-projection runs in fp8: host supplies x and Wq quantized to e4m3
(scales 16 and 4096) in the DoubleRow pair layout; measured end-to-end
error ~1.2e-2 vs the 2e-2 gate.
"""

import math
import sys

import numpy as np
import ml_dtypes

try:  # concourse ships in the container; fall back to the repo checkout
    import concourse.bass  # noqa: F401
except ImportError:  # pragma: no cover
    for p in ("/opt/trn_rl_repo", "/root/.axon_site/_ro/trn_rl_repo"):
        if p not in sys.path:
            sys.path.insert(0, p)

# Problem constants (hardcoded; harness calls kernel() standalone).
D_MODEL = 2048
N_HEADS = 16
R = 512          # kv lora rank
DN = 128         # d_nope
DR = 64          # d_rope
DV = 128         # d_v
ROPE_THETA = 10000.0
B = 2
S = 2048
HP = 4           # heads per core
QB = 512         # q block size
NKC = S // 128   # 16 k chunks
NQB = S // QB    # 4 q blocks
NCORES = 8

BF16 = ml_dtypes.bfloat16
F8 = ml_dtypes.float8_e4m3fn
SX = 16.0        # fp8 scale on x
SW = 4096.0      # fp8 scale on (Wq * softmax_scale)
QSCALE = 1.0 / (SX * SW)

_PROGRAM = {}


def _build_program(split_waits=True):
    import concourse.bass as bass
    import concourse.mybir as mybir
    from concourse.tile import TileContext

    def split_multi_waits(max_waits=1):
        """The walrus build in this container rejects instructions with
        more than `max_waits` sync-wait commands. Move excess waits onto
        same-engine NoOps inserted just before the instruction."""
        for f in nc.m.functions:
            for bb in f.blocks:
                out = []
                changed = False
                for inst in bb.instructions:
                    si = getattr(inst, "sync_info", None)
                    ws = list(si.on_wait) if si is not None else []
                    if len(ws) > max_waits:
                        changed = True
                        inst.sync_info = mybir.SyncInfo(
                            on_wait=ws[:max_waits],
                            on_update=list(si.on_update))
                        for w in ws[max_waits:]:
                            n = mybir.InstNoOp(
                                name=nc.get_next_instruction_name(),
                                ins=[], outs=[])
                            n.engine = inst.engine
                            n.sync_info = mybir.SyncInfo(
                                on_wait=[w], on_update=[])
                            out.append(n)
                    out.append(inst)
                if changed:
                    bb.instructions = out

    f32 = mybir.dt.float32
    cdt = mybir.dt.bfloat16
    f8 = mybir.dt.float8e4
    DRMODE = mybir.MatmulPerfMode.DoubleRow

    nc = bass.Bass()

    xT = nc.dram_tensor("xT", [D_MODEL, S], cdt, kind="ExternalInput")
    # fp8 pair layout: row c*128+p holds [xT[256c+p, :] | xT[256c+128+p, :]]
    x8d = nc.dram_tensor("x8d", [D_MODEL // 2, 2 * S], f8, kind="ExternalInput")
    wq8d = nc.dram_tensor("wq8d", [D_MODEL // 2, 2 * HP * (DN + DR)], f8,
                          kind="ExternalInput")
    wkva = nc.dram_tensor("wkva", [D_MODEL, R + DR], cdt, kind="ExternalInput")
    wkvbk8 = nc.dram_tensor("wkvbk8", [R // 2, 2 * HP * DN], f8,
                            kind="ExternalInput")
    wkvbv = nc.dram_tensor("wkvbv", [R, HP * DV], cdt, kind="ExternalInput")
    wo = nc.dram_tensor("wo", [HP * DV, D_MODEL], cdt, kind="ExternalInput")
    cosf = nc.dram_tensor("cosf", [128, S], cdt, kind="ExternalInput")
    sinf = nc.dram_tensor("sinf", [128, S], cdt, kind="ExternalInput")
    masks = nc.dram_tensor("masks", [128, 128], cdt, kind="ExternalInput")
    ident = nc.dram_tensor("ident", [128, 128], cdt, kind="ExternalInput")
    ones = nc.dram_tensor("ones", [128, 1], cdt, kind="ExternalInput")
    onesf = nc.dram_tensor("onesf", [1, 128], cdt, kind="ExternalInput")
    outp = nc.dram_tensor("outp", [D_MODEL, S], cdt, kind="ExternalOutput")

    Exp = mybir.ActivationFunctionType.Exp
    Ln = mybir.ActivationFunctionType.Ln

    NMT = HP * (DN + DR) // 128  # 6 qT M-tiles

    with TileContext(nc) as tc:
        with (
            tc.tile_pool(name="const", bufs=1) as cpool,
            tc.tile_pool(name="persist", bufs=1) as ppool,
        ):
            cosf_sb = cpool.tile([128, S], cdt, name="cosf_sb")
            sinf_sb = cpool.tile([128, S], cdt, name="sinf_sb")
            masks_sb = cpool.tile([128, 128], cdt, name="masks_sb")
            ident_sb = cpool.tile([128, 128], cdt, name="ident_sb")
            ones_sb = cpool.tile([128, 1], cdt, name="ones_sb")
            onesb_sb = cpool.tile([1, 128], cdt, name="onesb_sb")

            # Persistent activations.
            qT = [
                ppool.tile([128, S], cdt, name=f"qT{m}", tag="qT", bufs=6)
                for m in range(6)
            ]
            ck = [
                ppool.tile([128, S], cdt, name=f"ck{m}", tag="cko", bufs=4)
                for m in range(4)
            ]
            kn = [
                ppool.tile([128, S], cdt, name=f"kn{m}", tag="kn", bufs=4)
                for m in range(4)
            ]
            kr = ppool.tile([128, S], cdt, name="krope", tag="krope", bufs=1)
            vt = [
                ppool.tile([128, HP * DV], cdt, name=f"v{i}", tag="v", bufs=NKC)
                for i in range(NKC)
            ]
            # RoPE swap scratch lives in the persistent pool so the kvT
            # weight pool does not WAR-serialize against the rope phase.
            swt = [
                ppool.tile([128, S], cdt, name=f"sw{i}", tag="sw", bufs=3)
                for i in range(3)
            ]

            # kv_b weights persist so their DMAs can issue at startup.
            wbk8_sb = [
                ppool.tile([128, 2, HP * DN], f8, name=f"wbk8_sb{p}",
                           tag="wbk8", bufs=2)
                for p in range(2)
            ]
            wbv_sb = [
                ppool.tile([128, HP * DV], cdt, name=f"wbv_sb{r}", tag="wbv",
                           bufs=4)
                for r in range(4)
            ]
            # fp8 copy of c_kvT in DoubleRow pair layout for the k_nope
            # up-projection (pair p holds chunks 2p, 2p+1); one contiguous
            # tile per (q-block, pair) so the matmul ifmap stream stays
            # contiguous.
            ck8p = [
                [
                    ppool.tile([128, 2, QB], f8, name=f"ck8p_{t}_{p}",
                               tag="ck8", bufs=8)
                    for p in range(2)
                ]
                for t in range(NQB)
            ]

            # ---- Phase 1: x projections, then kv up-projection ----
            with (
                tc.tile_pool(name="wproj", bufs=1) as wpool,
                tc.tile_pool(name="xstream", bufs=1) as xpool,
                tc.tile_pool(name="psA", bufs=8, space="PSUM") as psA,
            ):
                # Quarter-0 DMAs, interleaved across four queues so the
                # first matmuls can start early: fp8 x pairs on gpsimd,
                # bf16 x chunks on sync, fp8 wq on scalar, wkva on vector.
                wq8_sb = []
                wkva_sb = []
                xq0 = []
                x80 = []
                NM = HP * (DN + DR)
                for c in range(8):
                    w8 = wpool.tile([128, 2, NM], f8,
                                    name=f"wq8_{c}", tag="wq8", bufs=8)
                    src3 = wq8d[c * 128:(c + 1) * 128, :].rearrange(
                        "p (two m) -> p two m", two=2)
                    if c == 0:
                        # Split the first weight load so the fp8 ladder can
                        # start after half the tile lands.
                        nc.scalar.dma_start(w8[:, :, 0:NM // 2],
                                            src3[:, :, 0:NM // 2])
                        nc.scalar.dma_start(w8[:, :, NM // 2:NM],
                                            src3[:, :, NM // 2:NM])
                    else:
                        nc.scalar.dma_start(w8, src3)
                    wq8_sb.append(w8)
                    x8t = xpool.tile([128, 2, QB], f8, name=f"x8_0_{c}",
                                     tag="x8", bufs=12)
                    src = x8d[c * 128:(c + 1) * 128, :].rearrange(
                        "p (two s) -> p two s", two=2)[:, :, 0:QB]
                    nc.scalar.dma_start(x8t, src)
                    x80.append(x8t)
                    for k in (2 * c, 2 * c + 1):
                        xk = xpool.tile([128, QB], cdt, name=f"xq_0_{k}",
                                        tag="xq0", bufs=16)
                        nc.sync.dma_start(xk, xT[k * 128:(k + 1) * 128, 0:QB])
                        xq0.append(xk)
                        w2 = wpool.tile([128, R + DR], cdt,
                                        name=f"wkva_sb{k}", tag="wkva",
                                        bufs=16)
                        nc.gpsimd.dma_start(w2, wkva[k * 128:(k + 1) * 128, :])
                        wkva_sb.append(w2)
                for p in range(2):
                    nc.sync.dma_start(
                        wbk8_sb[p].rearrange("p two m -> p (two m)"),
                        wkvbk8[p * 128:(p + 1) * 128, :])
                for r in range(4):
                    nc.sync.dma_start(wbv_sb[r], wkvbv[r * 128:(r + 1) * 128, :])
                nc.gpsimd.dma_start(cosf_sb, cosf[:, :])
                nc.gpsimd.dma_start(sinf_sb, sinf[:, :])
                nc.gpsimd.dma_start(masks_sb, masks[:, :])
                nc.gpsimd.dma_start(ident_sb, ident[:, :])
                nc.gpsimd.dma_start(ones_sb, ones[:, :])
                nc.gpsimd.dma_start(onesb_sb, onesf[:, :])

                # Quarter 0 is DMA-latency bound: run the contraction OUTER
                # over 8 PSUM banks (6 qT fp8 ladders + ck0/ck1 bf16) so
                # each arriving x chunk gets work immediately.
                t0 = slice(0, QB)
                ps8 = [
                    psA.tile([128, QB], f32, name=f"psq0_{m}", tag="ps")
                    for m in range(8)
                ]
                for s in range(16):
                    for m in range(2):
                        nc.tensor.matmul(
                            ps8[6 + m], lhsT=wkva_sb[s][:, m * 128:(m + 1) * 128],
                            rhs=xq0[s], start=(s == 0), stop=(s == 15))
                    if s % 2 == 1:
                        c = s // 2
                        for m in range(NMT):
                            nc.tensor.matmul(
                                ps8[m],
                                lhsT=wq8_sb[c][:, :, m * 128:(m + 1) * 128],
                                rhs=x80[c], start=(c == 0), stop=(c == 7),
                                perf_mode=DRMODE)
                for m in range(6):
                    nc.scalar.mul(qT[m][:, t0], ps8[m], QSCALE)
                for m in range(2):
                    nc.vector.tensor_copy(ck[m][:, t0], ps8[6 + m])
                    nc.vector.tensor_scalar_mul(
                        ck8p[0][m // 2][:, m % 2, :], ck[m][:, t0], SX)
                for m in (2, 3):
                    ps = psA.tile([128, QB], f32, name=f"psk_0_{m}", tag="ps")
                    for k in range(16):
                        nc.tensor.matmul(
                            ps, lhsT=wkva_sb[k][:, m * 128:(m + 1) * 128],
                            rhs=xq0[k], start=(k == 0), stop=(k == 15))
                    nc.vector.tensor_copy(ck[m][:, t0], ps)
                    nc.vector.tensor_scalar_mul(
                        ck8p[0][m // 2][:, m % 2, :], ck[m][:, t0], SX)
                ps = psA.tile([64, QB], f32, name="psr_0", tag="ps")
                for k in range(16):
                    nc.tensor.matmul(
                        ps, lhsT=wkva_sb[k][:, R:R + DR],
                        rhs=xq0[k], start=(k == 0), stop=(k == 15))
                nc.scalar.copy(kr[0:64, t0], ps)
                nc.scalar.copy(kr[64:128, t0], ps)

                for t in range(1, NQB):
                    tcols = slice(t * QB, (t + 1) * QB)
                    # x DMAs for this block: 4 batched bf16 + 8 fp8 pairs.
                    xqb = []
                    for g in range(4):
                        xb = xpool.tile([128, 4, QB], cdt, name=f"xqb_{t}_{g}",
                                        tag="xqb", bufs=6)
                        src = xT[:, tcols].rearrange(
                            "(c p) s -> p c s", p=128)[:, 4 * g:4 * g + 4, :]
                        nc.sync.dma_start(xb, src)
                        xqb.append(xb)
                    x8b = []
                    for c in range(8):
                        x8t = xpool.tile([128, 2, QB], f8, name=f"x8_{t}_{c}",
                                         tag="x8", bufs=12)
                        src = x8d[c * 128:(c + 1) * 128, :].rearrange(
                            "p (two s) -> p two s", two=2)[:, :, tcols]
                        nc.gpsimd.dma_start(x8t, src)
                        x8b.append(x8t)

                    def xqc(k):
                        return xqb[k // 4][:, k % 4, :]

                    # qT M-tiles (fp8 DoubleRow, 8 contraction steps)
                    for m in range(NMT):
                        ps = psA.tile([128, QB], f32, name=f"psq_{t}_{m}",
                                      tag="ps")
                        for c in range(8):
                            nc.tensor.matmul(
                                ps, lhsT=wq8_sb[c][:, :, m * 128:(m + 1) * 128],
                                rhs=x8b[c], start=(c == 0), stop=(c == 7),
                                perf_mode=DRMODE)
                        nc.scalar.mul(qT[m][:, tcols], ps, QSCALE)
                    # c_kvT M-tiles (bf16)
                    for m in range(4):
                        ps = psA.tile([128, QB], f32, name=f"psk_{t}_{m}",
                                      tag="ps")
                        for k in range(16):
                            nc.tensor.matmul(
                                ps, lhsT=wkva_sb[k][:, m * 128:(m + 1) * 128],
                                rhs=xqc(k), start=(k == 0), stop=(k == 15))
                        nc.vector.tensor_copy(ck[m][:, tcols], ps)
                        nc.vector.tensor_scalar_mul(
                            ck8p[t][m // 2][:, m % 2, :], ck[m][:, tcols], SX)
                    # k_ropeT (rows 512..575 of kv_aT), duplicated into kr
                    ps = psA.tile([64, QB], f32, name=f"psr_{t}", tag="ps")
                    for k in range(16):
                        nc.tensor.matmul(
                            ps, lhsT=wkva_sb[k][:, R:R + DR],
                            rhs=xqc(k), start=(k == 0), stop=(k == 15))
                    nc.scalar.copy(kr[0:64, tcols], ps)
                    nc.scalar.copy(kr[64:128, tcols], ps)

                # ---- RoPE rotation (in place; DVE work overlaps the
                # kv up-projection matmuls below) ----
                # rot = x * cosf + swap32(x) * sinf, where swap32 swaps each
                # 32-row half within every 64-row group (signs in sinf).
                for idx, tap in enumerate([qT[4], qT[5], kr]):
                    sw = swt[idx]
                    for blk in range(4):
                        src = (blk ^ 1) * 32
                        nc.sync.dma_start(
                            sw[blk * 32:(blk + 1) * 32, :],
                            tap[src:src + 32, :])
                    nc.vector.tensor_mul(tap, tap, cosf_sb)
                    nc.vector.tensor_mul(sw, sw, sinf_sb)
                    nc.vector.tensor_add(tap, tap, sw)

                # ---- kv up-projection (k_nopeT, v); same PSUM pool ----
                # k_nope runs in fp8 DoubleRow (K=256 per step).
                for m in range(4):
                    for nb in range(NQB):
                        ncols = slice(nb * QB, (nb + 1) * QB)
                        ps = psA.tile([128, QB], f32, name=f"psn_{m}_{nb}",
                                      tag="ps")
                        for p in range(2):
                            nc.tensor.matmul(
                                ps, lhsT=wbk8_sb[p][:, :, m * 128:(m + 1) * 128],
                                rhs=ck8p[nb][p], start=(p == 0),
                                stop=(p == 1), perf_mode=DRMODE)
                        nc.scalar.mul(kn[m][:, ncols], ps, 1.0 / (SX * 512.0))
                for i in range(NKC):
                    ps = psA.tile([128, HP * DV], f32, name=f"psv_{i}", tag="ps")
                    for r in range(4):
                        nc.tensor.matmul(
                            ps, lhsT=ck[r][:, i * 128:(i + 1) * 128],
                            rhs=wbv_sb[r], start=(r == 0), stop=(r == 3))
                    nc.vector.tensor_copy(vt[i], ps)

            # outT tiles reuse the c_kvT slots (same tag, 4 bufs).
            outT = [
                ppool.tile([128, S], cdt, name=f"outT{h}", tag="cko", bufs=4)
                for h in range(HP)
            ]

            # ---- Phase 3: attention + output projection ----
            with (
                tc.tile_pool(name="att", bufs=1) as apool,
                tc.tile_pool(name="psS", bufs=4, space="PSUM") as psS,
                tc.tile_pool(name="psO", bufs=2, space="PSUM") as psO,
                tc.tile_pool(name="psD", bufs=1, space="PSUM") as psD,
                tc.tile_pool(name="psBC", bufs=1, space="PSUM") as psBC,
            ):
                # Wo loads overlap the attention phase on the idle sync queue.
                wo_sb = [
                    apool.tile([128, D_MODEL], cdt, name=f"wo_sb{r}", tag="wo",
                               bufs=4)
                    for r in range(4)
                ]
                for r in range(4):
                    nc.sync.dma_start(wo_sb[r], wo[r * 128:(r + 1) * 128, :])

                pend_den = None   # (h, j, accb, ops)
                pend_norm = None  # (h, j, ops, recb)

                def emit_den(h, j, accb, ops):
                    # Single ones-matmul over the DVE-accumulated exp sums,
                    # then 1/denom as exp(-ln(d)) on the ACT engine.
                    nonlocal pend_norm
                    dps = psD.tile([1, QB], f32, name=f"dps_{h}_{j}", tag="d")
                    nc.tensor.matmul(dps, lhsT=ones_sb, rhs=accb,
                                     start=True, stop=True)
                    rec = apool.tile([1, QB], f32, name=f"rec_{h}_{j}",
                                     tag="rec", bufs=2)
                    nc.scalar.activation(rec, dps, Ln)
                    recb = apool.tile([1, QB], cdt, name=f"recb_{h}_{j}",
                                      tag="recb", bufs=2)
                    nc.scalar.activation(recb, rec, Exp, scale=-1.0)
                    pend_norm = (h, j, ops, recb)

                def norm_late(h, j, ops, recb):
                    # Broadcast 1/denom across partitions via a K=1 matmul,
                    # then scale the out accumulator into outT.
                    qs = slice(j * QB, (j + 1) * QB)
                    bps = psBC.tile([128, QB], f32, name=f"bps_{h}_{j}",
                                    tag="b")
                    nc.tensor.matmul(bps, lhsT=onesb_sb, rhs=recb,
                                     start=True, stop=True)
                    bc = apool.tile([128, QB], f32, name=f"bc_{h}_{j}",
                                    tag="bc", bufs=2)
                    nc.scalar.copy(bc, bps)
                    nc.vector.tensor_mul(outT[h][:, qs], ops, bc)

                def emit_wo_block(nb):
                    # One 512-column block of the output projection; all 16
                    # M-tiles. Emitted as soon as every head's outT for this
                    # block is normalized, so the projection overlaps the
                    # tail of the attention phase.
                    ncols = slice(nb * QB, (nb + 1) * QB)
                    for m in range(16):
                        wopool, wotag = (psS, "s") if m % 2 == 0 else (psO, "o")
                        ps = wopool.tile([128, QB], f32, name=f"psw_{m}_{nb}",
                                         tag=wotag)
                        for r in range(4):
                            nc.tensor.matmul(
                                ps, lhsT=wo_sb[r][:, m * 128:(m + 1) * 128],
                                rhs=outT[r][:, ncols], start=(r == 0),
                                stop=(r == 3))
                        st = apool.tile([128, QB], cdt, name=f"st_{m}_{nb}",
                                        tag="st", bufs=4)
                        if m % 2 == 0:
                            nc.scalar.copy(st, ps)
                        else:
                            nc.vector.tensor_copy(st, ps)
                        dma_eng = (nc.sync, nc.scalar, nc.gpsimd)[m % 3]
                        dma_eng.dma_start(
                            outp[m * 128:(m + 1) * 128, ncols], st)

                # Descending j per head: every normalize chain (DVE sums ->
                # ones-matmul -> Ln/Exp -> bcast) then hides inside a large
                # (12-16 chunk) follower group instead of a 4-chunk one.
                for h in range(HP):
                    qn = qT[h]
                    qr = qT[4 + h // 2]
                    off = (h % 2) * 64
                    for j in range(NQB - 1, -1, -1):
                        qs = slice(j * QB, (j + 1) * QB)
                        ops = psO.tile([128, QB], f32, name=f"ops_{h}_{j}",
                                       tag="o")
                        acc = apool.tile([128, QB], f32, name=f"acc_{h}_{j}",
                                         tag="acc", bufs=2)
                        nch = 4 * (j + 1)
                        for c in range(nch):
                            ks = slice(c * 128, (c + 1) * 128)
                            r = c - 4 * j
                            # Diagonal chunks only need columns >= r*128
                            # (everything to the left is strictly above the
                            # causal boundary). Chunk 0 always start-covers
                            # the full accumulator width.
                            col0 = max(0, r * 128)
                            w = slice(col0, QB)
                            qsw = slice(j * QB + col0, (j + 1) * QB)
                            sps = psS.tile([128, QB], f32,
                                           name=f"sps_{h}_{j}_{c}", tag="s")
                            nc.tensor.matmul(sps[:, w], lhsT=kn[h][:, ks],
                                             rhs=qn[:, qsw], start=True,
                                             stop=False,
                                             skip_group_check=True)
                            nc.tensor.matmul(sps[:, w],
                                             lhsT=kr[off:off + 64, ks],
                                             rhs=qr[off:off + 64, qsw],
                                             start=False, stop=(r < 0),
                                             skip_group_check=True)
                            if r >= 0:
                                # Add the causal tri mask on the PE itself
                                # (identity @ tri) so exp never waits on a
                                # cross-engine DVE hop.
                                nc.tensor.matmul(
                                    sps[:, col0:col0 + 128], lhsT=ident_sb,
                                    rhs=masks_sb, start=False, stop=True,
                                    skip_group_check=True)
                            pt = apool.tile([128, QB], cdt,
                                            name=f"pt_{h}_{j}_{c}", tag="pt",
                                            bufs=4)
                            nc.scalar.activation(pt[:, w], sps[:, w], Exp)
                            nc.tensor.matmul(
                                ops[:, w], lhsT=vt[c][:, h * DV:(h + 1) * DV],
                                rhs=pt[:, w], start=(c == 0),
                                stop=(c == nch - 1), skip_group_check=True)
                            if c == 0:
                                nc.vector.tensor_copy(acc, pt)
                            else:
                                nc.vector.tensor_add(acc[:, w], acc[:, w],
                                                     pt[:, w])
                            if c == 1 and pend_den is not None:
                                emit_den(*pend_den)
                                pend_den = None
                            if c == 3 and pend_norm is not None:
                                norm_late(*pend_norm)
                                pend_norm = None
                        accb = apool.tile([128, QB], cdt,
                                          name=f"accb_{h}_{j}", tag="accb",
                                          bufs=2)
                        nc.vector.tensor_copy(accb, acc)
                        pend_den = (h, j, accb, ops)
                        if h == HP - 1 and j < NQB - 1:
                            # outT[:, block j+1] is normalized for all heads
                            # by now; project it while attention continues.
                            emit_wo_block(j + 1)
                emit_den(*pend_den)
                norm_late(*pend_norm)
                emit_wo_block(0)

                # ---- Output projection; PSUM reuses the score slots ----


    if split_waits:
        split_multi_waits()
    return nc


def get_program(split_waits=True):
    if split_waits not in _PROGRAM:
        _PROGRAM[split_waits] = _build_program(split_waits)
    return _PROGRAM[split_waits]


def make_core_inputs(x, Wq, Wkv_a, Wkv_b, Wo):
    """Host-side sharding/pre-processing. Returns list of 8 input dicts."""
    scale = 1.0 / math.sqrt(DN + DR)

    inv_freq = 1.0 / (ROPE_THETA ** (np.arange(0, DR, 2, dtype=np.float64) / DR))
    t = np.arange(S, dtype=np.float64)
    freqs = np.outer(t, inv_freq)                      # [S, 32]
    cos32 = np.cos(freqs).T.astype(np.float32)         # [32, S]
    sin32 = np.sin(freqs).T.astype(np.float32)
    cosf = np.tile(cos32, (4, 1)).astype(BF16)         # [128, S]
    sinf = np.tile(np.concatenate([-sin32, sin32], axis=0), (2, 1)).astype(BF16)

    row = np.arange(128)[:, None]
    col = np.arange(128)[None, :]
    masks = np.where(col >= row, 0.0, -1e30).astype(BF16)  # [128, 128]
    ident = np.eye(128, dtype=BF16)
    ones = np.ones([128, 1], dtype=BF16)
    onesf = np.ones([1, 128], dtype=BF16)

    Wq_r = np.asarray(Wq, dtype=np.float32).reshape(D_MODEL, N_HEADS, DN + DR)
    Wb_r = np.asarray(Wkv_b, dtype=np.float32).reshape(R, N_HEADS, DN + DV)
    Wo_f = np.asarray(Wo, dtype=np.float32)
    Wkva_f = np.asarray(Wkv_a, dtype=np.float32).astype(BF16)
    x_f = np.asarray(x, dtype=np.float32)

    def pair_rows(a):
        """[2K, N] -> [K, 2N] with row c*128+p = [a[256c+p] | a[256c+128+p]]."""
        k2, n = a.shape
        out = np.empty((k2 // 2, 2 * n), dtype=a.dtype)
        for c in range(k2 // 256):
            out[c * 128:(c + 1) * 128, :n] = a[256 * c:256 * c + 128]
            out[c * 128:(c + 1) * 128, n:] = a[256 * c + 128:256 * c + 256]
        return out

    in_maps = []
    x8_cache = {}
    for c in range(NCORES):
        b, g = divmod(c, HP)
        heads = list(range(HP * g, HP * g + HP))
        xTc = np.ascontiguousarray(x_f[b].T)
        if b not in x8_cache:
            x8_cache[b] = np.ascontiguousarray(
                pair_rows((xTc * SX).astype(F8)))
        wq_nope = Wq_r[:, heads, :DN].reshape(D_MODEL, HP * DN)
        wq_rope = Wq_r[:, heads, DN:].reshape(D_MODEL, HP * DR)
        wq_c = np.concatenate([wq_nope, wq_rope], axis=1) * scale
        wq8_c = pair_rows((wq_c * SW).astype(F8))
        wbk8_c = pair_rows(
            (Wb_r[:, heads, :DN].reshape(R, HP * DN) * 512.0).astype(F8))
        wbv_c = np.ascontiguousarray(
            Wb_r[:, heads, DN:].reshape(R, HP * DV)).astype(BF16)
        wo_c = np.ascontiguousarray(
            Wo_f[HP * g * DV:(HP * g + HP) * DV, :]).astype(BF16)
        in_maps.append({
            "xT": xTc.astype(BF16),
            "x8d": x8_cache[b],
            "wq8d": np.ascontiguousarray(wq8_c),
            "wkva": Wkva_f,
            "wkvbk8": np.ascontiguousarray(wbk8_c),
            "wkvbv": wbv_c,
            "wo": wo_c,
            "cosf": cosf,
            "sinf": sinf,
            "masks": masks,
            "ident": ident,
            "ones": ones,
            "onesf": onesf,
        })
    return in_maps


def gather_output(results):
    """results: list of 8 dicts with 'outp' [D_MODEL, S] bf16 partials."""
    out = np.empty((B, S, D_MODEL), dtype=np.float32)
    for b in range(B):
        acc = results[HP * b]["outp"].astype(np.float32)
        for g in range(1, HP):
            acc = acc + results[HP * b + g]["outp"].astype(np.float32)
        out[b] = acc.T
    return out


def kernel(x, Wq, Wkv_a, Wkv_b, Wo):
    from concourse.bass_utils import run_bass_kernel_spmd

    nc = get_program()
    in_maps = make_core_inputs(x, Wq, Wkv_a, Wkv_b, Wo)
    res = run_bass_kernel_spmd(nc, in_maps, list(range(NCORES)))
    return gather_output(res.results)


# revision 25
# speedup vs baseline: 1.0067x; 1.0067x over previous
"""Multi-Head Latent Attention (MLA) TRN2 Bass kernel.

Sharding: data-parallel over batch (B=2) x tensor-parallel over heads
(16 heads -> 4 per core) = 8 cores. The kv_lora latent path and shared
rope key are computed replicated within each batch group; the final
output projection is computed as per-core partials which the host sums.

All on-device dataflow is "transposed" (feature dim on partitions,
sequence on the free dim) so no PE transposes are ever needed:
  qT      = Wq_perm^T @ xT          [768, S]   via fp8-e4m3 DoubleRow
                                               matmuls (K=256/step, 2x rate)
  kv_aT   = Wkv_a^T @ xT            [576, S]   bf16 (c_kvT rows 0..511,
                                               k_ropeT rows 512..575)
  k_nopeT = Wkv_b_k^T @ c_kvT       [512, S]
  v       = (c_kvT chunk)^T-matmuls [S, 512]   (natural layout)
  RoPE applied in transposed layout with a partition-swap DMA + 3 DVE ops
  scoresT[s_k, s_q] per (head, q-block of 512), causal masks added on the
  4 diagonal chunks, exp on ACT (no max subtraction; scores are bounded),
  softmax denominators via DVE accumulation of the exp tiles + a single
  ones-matmul per (head, q-block), out^T accumulated in PSUM, normalized
  by broadcasted reciprocals, then partialT = Wo_c^T @ outT in bf16.

The q-projection runs in fp8: host supplies x and Wq quantized to e4m3
(scales 16 and 4096) in the DoubleRow pair layout; measured end-to-end
error ~1.2e-2 vs the 2e-2 gate.
"""

import math
import sys

import numpy as np
import ml_dtypes

try:  # concourse ships in the container; fall back to the repo checkout
    import concourse.bass  # noqa: F401
except ImportError:  # pragma: no cover
    for p in ("/opt/trn_rl_repo", "/root/.axon_site/_ro/trn_rl_repo"):
        if p not in sys.path:
            sys.path.insert(0, p)

# Problem constants (hardcoded; harness calls kernel() standalone).
D_MODEL = 2048
N_HEADS = 16
R = 512          # kv lora rank
DN = 128         # d_nope
DR = 64          # d_rope
DV = 128         # d_v
ROPE_THETA = 10000.0
B = 2
S = 2048
HP = 4           # heads per core
QB = 512         # q block size
NKC = S // 128   # 16 k chunks
NQB = S // QB    # 4 q blocks
NCORES = 8

BF16 = ml_dtypes.bfloat16
F8 = ml_dtypes.float8_e4m3fn
SX = 16.0        # fp8 scale on x
SW = 4096.0      # fp8 scale on (Wq * softmax_scale)
QSCALE = 1.0 / (SX * SW)

_PROGRAM = {}


def _build_program(split_waits=True):
    import concourse.bass as bass
    import concourse.mybir as mybir
    from concourse.tile import TileContext

    def split_multi_waits(max_waits=1):
        """The walrus build in this container rejects instructions with
        more than `max_waits` sync-wait commands. Move excess waits onto
        same-engine NoOps inserted just before the instruction."""
        for f in nc.m.functions:
            for bb in f.blocks:
                out = []
                changed = False
                for inst in bb.instructions:
                    si = getattr(inst, "sync_info", None)
                    ws = list(si.on_wait) if si is not None else []
                    if len(ws) > max_waits:
                        changed = True
                        inst.sync_info = mybir.SyncInfo(
                            on_wait=ws[:max_waits],
                            on_update=list(si.on_update))
                        for w in ws[max_waits:]:
                            n = mybir.InstNoOp(
                                name=nc.get_next_instruction_name(),
                                ins=[], outs=[])
                            n.engine = inst.engine
                            n.sync_info = mybir.SyncInfo(
                                on_wait=[w], on_update=[])
                            out.append(n)
                    out.append(inst)
                if changed:
                    bb.instructions = out

    f32 = mybir.dt.float32
    cdt = mybir.dt.bfloat16
    f8 = mybir.dt.float8e4
    DRMODE = mybir.MatmulPerfMode.DoubleRow

    nc = bass.Bass()

    xT = nc.dram_tensor("xT", [D_MODEL, S], cdt, kind="ExternalInput")
    # fp8 pair layout: row c*128+p holds [xT[256c+p, :] | xT[256c+128+p, :]]
    x8d = nc.dram_tensor("x8d", [D_MODEL // 2, 2 * S], f8, kind="ExternalInput")
    wq8d = nc.dram_tensor("wq8d", [D_MODEL // 2, 2 * HP * (DN + DR)], f8,
                          kind="ExternalInput")
    wkva = nc.dram_tensor("wkva", [D_MODEL, R + DR], cdt, kind="ExternalInput")
    wkvbk8 = nc.dram_tensor("wkvbk8", [R // 2, 2 * HP * DN], f8,
                            kind="ExternalInput")
    wkvbv = nc.dram_tensor("wkvbv", [R, HP * DV], cdt, kind="ExternalInput")
    wo = nc.dram_tensor("wo", [HP * DV, D_MODEL], cdt, kind="ExternalInput")
    cosf = nc.dram_tensor("cosf", [128, S], cdt, kind="ExternalInput")
    sinf = nc.dram_tensor("sinf", [128, S], cdt, kind="ExternalInput")
    masks = nc.dram_tensor("masks", [128, 128], cdt, kind="ExternalInput")
    ident = nc.dram_tensor("ident", [128, 128], cdt, kind="ExternalInput")
    ones = nc.dram_tensor("ones", [128, 1], cdt, kind="ExternalInput")
    onesf = nc.dram_tensor("onesf", [1, 128], cdt, kind="ExternalInput")
    outp = nc.dram_tensor("outp", [D_MODEL, S], cdt, kind="ExternalOutput")

    Exp = mybir.ActivationFunctionType.Exp
    Ln = mybir.ActivationFunctionType.Ln

    NMT = HP * (DN + DR) // 128  # 6 qT M-tiles

    with TileContext(nc) as tc:
        with (
            tc.tile_pool(name="const", bufs=1) as cpool,
            tc.tile_pool(name="persist", bufs=1) as ppool,
        ):
            cosf_sb = cpool.tile([128, S], cdt, name="cosf_sb")
            sinf_sb = cpool.tile([128, S], cdt, name="sinf_sb")
            masks_sb = cpool.tile([128, 128], cdt, name="masks_sb")
            ident_sb = cpool.tile([128, 128], cdt, name="ident_sb")
            ones_sb = cpool.tile([128, 1], cdt, name="ones_sb")
            onesb_sb = cpool.tile([1, 128], cdt, name="onesb_sb")

            # Persistent activations.
            qT = [
                ppool.tile([128, S], cdt, name=f"qT{m}", tag="qT", bufs=6)
                for m in range(6)
            ]
            ck = [
                ppool.tile([128, S], cdt, name=f"ck{m}", tag="cko", bufs=4)
                for m in range(4)
            ]
            kn = [
                ppool.tile([128, S], cdt, name=f"kn{m}", tag="kn", bufs=4)
                for m in range(4)
            ]
            kr = ppool.tile([128, S], cdt, name="krope", tag="krope", bufs=1)
            vt = [
                ppool.tile([128, HP * DV], cdt, name=f"v{i}", tag="v", bufs=NKC)
                for i in range(NKC)
            ]
            # RoPE swap scratch lives in the persistent pool so the kvT
            # weight pool does not WAR-serialize against the rope phase.
            swt = [
                ppool.tile([128, S], cdt, name=f"sw{i}", tag="sw", bufs=3)
                for i in range(3)
            ]

            # kv_b weights persist so their DMAs can issue at startup.
            wbk8_sb = [
                ppool.tile([128, 2, HP * DN], f8, name=f"wbk8_sb{p}",
                           tag="wbk8", bufs=2)
                for p in range(2)
            ]
            wbv_sb = [
                ppool.tile([128, HP * DV], cdt, name=f"wbv_sb{r}", tag="wbv",
                           bufs=4)
                for r in range(4)
            ]
            # fp8 copy of c_kvT in DoubleRow pair layout for the k_nope
            # up-projection (pair p holds chunks 2p, 2p+1); one contiguous
            # tile per (q-block, pair) so the matmul ifmap stream stays
            # contiguous.
            ck8p = [
                [
                    ppool.tile([128, 2, QB], f8, name=f"ck8p_{t}_{p}",
                               tag="ck8", bufs=8)
                    for p in range(2)
                ]
                for t in range(NQB)
            ]

            # ---- Phase 1: x projections, then kv up-projection ----
            with (
                tc.tile_pool(name="wproj", bufs=1) as wpool,
                tc.tile_pool(name="xstream", bufs=1) as xpool,
                tc.tile_pool(name="psA", bufs=8, space="PSUM") as psA,
            ):
                # Quarter-0 DMAs, interleaved across four queues so the
                # first matmuls can start early: fp8 x pairs on gpsimd,
                # bf16 x chunks on sync, fp8 wq on scalar, wkva on vector.
                wq8_sb = []
                wkva_sb = []
                xq0 = []
                x80 = []
                NM = HP * (DN + DR)
                for c in range(8):
                    w8 = wpool.tile([128, 2, NM], f8,
                                    name=f"wq8_{c}", tag="wq8", bufs=8)
                    src3 = wq8d[c * 128:(c + 1) * 128, :].rearrange(
                        "p (two m) -> p two m", two=2)
                    if c == 0:
                        # Split the first weight load so the fp8 ladder can
                        # start after half the tile lands.
                        nc.scalar.dma_start(w8[:, :, 0:NM // 2],
                                            src3[:, :, 0:NM // 2])
                        nc.scalar.dma_start(w8[:, :, NM // 2:NM],
                                            src3[:, :, NM // 2:NM])
                    else:
                        nc.scalar.dma_start(w8, src3)
                    wq8_sb.append(w8)
                    x8t = xpool.tile([128, 2, QB], f8, name=f"x8_0_{c}",
                                     tag="x8", bufs=12)
                    src = x8d[c * 128:(c + 1) * 128, :].rearrange(
                        "p (two s) -> p two s", two=2)[:, :, 0:QB]
                    nc.scalar.dma_start(x8t, src)
                    x80.append(x8t)
                    for k in (2 * c, 2 * c + 1):
                        xk = xpool.tile([128, QB], cdt, name=f"xq_0_{k}",
                                        tag="xq0", bufs=16)
                        nc.sync.dma_start(xk, xT[k * 128:(k + 1) * 128, 0:QB])
                        xq0.append(xk)
                        w2 = wpool.tile([128, R + DR], cdt,
                                        name=f"wkva_sb{k}", tag="wkva",
                                        bufs=16)
                        nc.gpsimd.dma_start(w2, wkva[k * 128:(k + 1) * 128, :])
                        wkva_sb.append(w2)
                for p in range(2):
                    nc.sync.dma_start(
                        wbk8_sb[p].rearrange("p two m -> p (two m)"),
                        wkvbk8[p * 128:(p + 1) * 128, :])
                for r in range(4):
                    nc.sync.dma_start(wbv_sb[r], wkvbv[r * 128:(r + 1) * 128, :])
                nc.gpsimd.dma_start(cosf_sb, cosf[:, :])
                nc.gpsimd.dma_start(sinf_sb, sinf[:, :])
                nc.gpsimd.dma_start(masks_sb, masks[:, :])
                nc.gpsimd.dma_start(ident_sb, ident[:, :])
                nc.gpsimd.dma_start(ones_sb, ones[:, :])
                nc.gpsimd.dma_start(onesb_sb, onesf[:, :])

                # Quarter 0 is DMA-latency bound: run the contraction OUTER
                # over 8 PSUM banks (6 qT fp8 ladders + ck0/ck1 bf16) so
                # each arriving x chunk gets work immediately.
                t0 = slice(0, QB)
                ps8 = [
                    psA.tile([128, QB], f32, name=f"psq0_{m}", tag="ps")
                    for m in range(8)
                ]
                for s in range(16):
                    for m in range(2):
                        nc.tensor.matmul(
                            ps8[6 + m], lhsT=wkva_sb[s][:, m * 128:(m + 1) * 128],
                            rhs=xq0[s], start=(s == 0), stop=(s == 15))
                    if s % 2 == 1:
                        c = s // 2
                        for m in range(NMT):
                            nc.tensor.matmul(
                                ps8[m],
                                lhsT=wq8_sb[c][:, :, m * 128:(m + 1) * 128],
                                rhs=x80[c], start=(c == 0), stop=(c == 7),
                                perf_mode=DRMODE)
                for m in range(6):
                    nc.scalar.mul(qT[m][:, t0], ps8[m], QSCALE)
                for m in range(2):
                    nc.vector.tensor_copy(ck[m][:, t0], ps8[6 + m])
                    nc.vector.tensor_scalar_mul(
                        ck8p[0][m // 2][:, m % 2, :], ck[m][:, t0], SX)
                for m in (2, 3):
                    ps = psA.tile([128, QB], f32, name=f"psk_0_{m}", tag="ps")
                    for k in range(16):
                        nc.tensor.matmul(
                            ps, lhsT=wkva_sb[k][:, m * 128:(m + 1) * 128],
                            rhs=xq0[k], start=(k == 0), stop=(k == 15))
                    nc.vector.tensor_copy(ck[m][:, t0], ps)
                    nc.vector.tensor_scalar_mul(
                        ck8p[0][m // 2][:, m % 2, :], ck[m][:, t0], SX)
                ps = psA.tile([64, QB], f32, name="psr_0", tag="ps")
                for k in range(16):
                    nc.tensor.matmul(
                        ps, lhsT=wkva_sb[k][:, R:R + DR],
                        rhs=xq0[k], start=(k == 0), stop=(k == 15))
                nc.scalar.copy(kr[0:64, t0], ps)
                nc.scalar.copy(kr[64:128, t0], ps)

                for t in range(1, NQB):
                    tcols = slice(t * QB, (t + 1) * QB)
                    # x DMAs for this block: 4 batched bf16 + 8 fp8 pairs.
                    xqb = []
                    for g in range(4):
                        xb = xpool.tile([128, 4, QB], cdt, name=f"xqb_{t}_{g}",
                                        tag="xqb", bufs=6)
                        src = xT[:, tcols].rearrange(
                            "(c p) s -> p c s", p=128)[:, 4 * g:4 * g + 4, :]
                        nc.sync.dma_start(xb, src)
                        xqb.append(xb)
                    x8b = []
                    for c in range(8):
                        x8t = xpool.tile([128, 2, QB], f8, name=f"x8_{t}_{c}",
                                         tag="x8", bufs=12)
                        src = x8d[c * 128:(c + 1) * 128, :].rearrange(
                            "p (two s) -> p two s", two=2)[:, :, tcols]
                        nc.gpsimd.dma_start(x8t, src)
                        x8b.append(x8t)

                    def xqc(k):
                        return xqb[k // 4][:, k % 4, :]

                    # Alternate fp8 qT ladders with bf16 c_kv ladders to
                    # smooth the PE power profile (fp8 DoubleRow doubles the
                    # MAC rate and can trip the utilization throttle).
                    def qt_ladder(m):
                        ps = psA.tile([128, QB], f32, name=f"psq_{t}_{m}",
                                      tag="ps")
                        for c in range(8):
                            nc.tensor.matmul(
                                ps, lhsT=wq8_sb[c][:, :, m * 128:(m + 1) * 128],
                                rhs=x8b[c], start=(c == 0), stop=(c == 7),
                                perf_mode=DRMODE)
                        nc.scalar.mul(qT[m][:, tcols], ps, QSCALE)

                    def ck_ladder(m):
                        ps = psA.tile([128, QB], f32, name=f"psk_{t}_{m}",
                                      tag="ps")
                        for k in range(16):
                            nc.tensor.matmul(
                                ps, lhsT=wkva_sb[k][:, m * 128:(m + 1) * 128],
                                rhs=xqc(k), start=(k == 0), stop=(k == 15))
                        nc.vector.tensor_copy(ck[m][:, tcols], ps)
                        nc.vector.tensor_scalar_mul(
                            ck8p[t][m // 2][:, m % 2, :], ck[m][:, tcols], SX)

                    def kr_ladder():
                        ps = psA.tile([64, QB], f32, name=f"psr_{t}", tag="ps")
                        for k in range(16):
                            nc.tensor.matmul(
                                ps, lhsT=wkva_sb[k][:, R:R + DR],
                                rhs=xqc(k), start=(k == 0), stop=(k == 15))
                        nc.scalar.copy(kr[0:64, tcols], ps)
                        nc.scalar.copy(kr[64:128, tcols], ps)

                    qt_ladder(0); ck_ladder(0); qt_ladder(1); ck_ladder(1)
                    qt_ladder(2); ck_ladder(2); qt_ladder(3); ck_ladder(3)
                    qt_ladder(4); kr_ladder(); qt_ladder(5)

                # ---- RoPE rotation (in place; DVE work overlaps the
                # kv up-projection matmuls below) ----
                # rot = x * cosf + swap32(x) * sinf, where swap32 swaps each
                # 32-row half within every 64-row group (signs in sinf).
                for idx, tap in enumerate([qT[4], qT[5], kr]):
                    sw = swt[idx]
                    for blk in range(4):
                        src = (blk ^ 1) * 32
                        nc.sync.dma_start(
                            sw[blk * 32:(blk + 1) * 32, :],
                            tap[src:src + 32, :])
                    nc.vector.tensor_mul(tap, tap, cosf_sb)
                    nc.vector.tensor_mul(sw, sw, sinf_sb)
                    nc.vector.tensor_add(tap, tap, sw)

                # ---- kv up-projection (k_nopeT, v); same PSUM pool ----
                # k_nope runs in fp8 DoubleRow (K=256 per step).
                for m in range(4):
                    for nb in range(NQB):
                        ncols = slice(nb * QB, (nb + 1) * QB)
                        ps = psA.tile([128, QB], f32, name=f"psn_{m}_{nb}",
                                      tag="ps")
                        for p in range(2):
                            nc.tensor.matmul(
                                ps, lhsT=wbk8_sb[p][:, :, m * 128:(m + 1) * 128],
                                rhs=ck8p[nb][p], start=(p == 0),
                                stop=(p == 1), perf_mode=DRMODE)
                        nc.scalar.mul(kn[m][:, ncols], ps, 1.0 / (SX * 512.0))
                for i in range(NKC):
                    ps = psA.tile([128, HP * DV], f32, name=f"psv_{i}", tag="ps")
                    for r in range(4):
                        nc.tensor.matmul(
                            ps, lhsT=ck[r][:, i * 128:(i + 1) * 128],
                            rhs=wbv_sb[r], start=(r == 0), stop=(r == 3))
                    nc.vector.tensor_copy(vt[i], ps)

            # outT tiles reuse the c_kvT slots (same tag, 4 bufs).
            outT = [
                ppool.tile([128, S], cdt, name=f"outT{h}", tag="cko", bufs=4)
                for h in range(HP)
            ]

            # ---- Phase 3: attention + output projection ----
            with (
                tc.tile_pool(name="att", bufs=1) as apool,
                tc.tile_pool(name="psS", bufs=4, space="PSUM") as psS,
                tc.tile_pool(name="psO", bufs=2, space="PSUM") as psO,
                tc.tile_pool(name="psD", bufs=1, space="PSUM") as psD,
                tc.tile_pool(name="psBC", bufs=1, space="PSUM") as psBC,
            ):
                # Wo loads overlap the attention phase on the idle sync queue.
                wo_sb = [
                    apool.tile([128, D_MODEL], cdt, name=f"wo_sb{r}", tag="wo",
                               bufs=4)
                    for r in range(4)
                ]
                for r in range(4):
                    nc.sync.dma_start(wo_sb[r], wo[r * 128:(r + 1) * 128, :])

                pend_den = None   # (h, j, accb, ops)
                pend_norm = None  # (h, j, ops, recb)

                def emit_den(h, j, accb, ops):
                    # Single ones-matmul over the DVE-accumulated exp sums,
                    # then 1/denom as exp(-ln(d)) on the ACT engine.
                    nonlocal pend_norm
                    dps = psD.tile([1, QB], f32, name=f"dps_{h}_{j}", tag="d")
                    nc.tensor.matmul(dps, lhsT=ones_sb, rhs=accb,
                                     start=True, stop=True)
                    rec = apool.tile([1, QB], f32, name=f"rec_{h}_{j}",
                                     tag="rec", bufs=2)
                    nc.scalar.activation(rec, dps, Ln)
                    recb = apool.tile([1, QB], cdt, name=f"recb_{h}_{j}",
                                      tag="recb", bufs=2)
                    nc.scalar.activation(recb, rec, Exp, scale=-1.0)
                    pend_norm = (h, j, ops, recb)

                def norm_late(h, j, ops, recb):
                    # Broadcast 1/denom across partitions via a K=1 matmul,
                    # then scale the out accumulator into outT.
                    qs = slice(j * QB, (j + 1) * QB)
                    bps = psBC.tile([128, QB], f32, name=f"bps_{h}_{j}",
                                    tag="b")
                    nc.tensor.matmul(bps, lhsT=onesb_sb, rhs=recb,
                                     start=True, stop=True)
                    bc = apool.tile([128, QB], f32, name=f"bc_{h}_{j}",
                                    tag="bc", bufs=2)
                    nc.scalar.copy(bc, bps)
                    nc.vector.tensor_mul(outT[h][:, qs], ops, bc)

                def emit_wo_block(nb):
                    # One 512-column block of the output projection; all 16
                    # M-tiles. Emitted as soon as every head's outT for this
                    # block is normalized, so the projection overlaps the
                    # tail of the attention phase.
                    ncols = slice(nb * QB, (nb + 1) * QB)
                    for m in range(16):
                        wopool, wotag = (psS, "s") if m % 2 == 0 else (psO, "o")
                        ps = wopool.tile([128, QB], f32, name=f"psw_{m}_{nb}",
                                         tag=wotag)
                        for r in range(4):
                            nc.tensor.matmul(
                                ps, lhsT=wo_sb[r][:, m * 128:(m + 1) * 128],
                                rhs=outT[r][:, ncols], start=(r == 0),
                                stop=(r == 3))
                        st = apool.tile([128, QB], cdt, name=f"st_{m}_{nb}",
                                        tag="st", bufs=4)
                        if m % 2 == 0:
                            nc.scalar.copy(st, ps)
                        else:
                            nc.vector.tensor_copy(st, ps)
                        dma_eng = (nc.sync, nc.scalar, nc.gpsimd)[m % 3]
                        dma_eng.dma_start(
                            outp[m * 128:(m + 1) * 128, ncols], st)

                # Descending j per head: every normalize chain (DVE sums ->
                # ones-matmul -> Ln/Exp -> bcast) then hides inside a large
                # (12-16 chunk) follower group instead of a 4-chunk one.
                for h in range(HP):
                    qn = qT[h]
                    qr = qT[4 + h // 2]
                    off = (h % 2) * 64
                    for j in range(NQB - 1, -1, -1):
                        qs = slice(j * QB, (j + 1) * QB)
                        ops = psO.tile([128, QB], f32, name=f"ops_{h}_{j}",
                                       tag="o")
                        acc = apool.tile([128, QB], f32, name=f"acc_{h}_{j}",
                                         tag="acc", bufs=2)
                        nch = 4 * (j + 1)
                        for c in range(nch):
                            ks = slice(c * 128, (c + 1) * 128)
                            r = c - 4 * j
                            # Diagonal chunks only need columns >= r*128
                            # (everything to the left is strictly above the
                            # causal boundary). Chunk 0 always start-covers
                            # the full accumulator width.
                            col0 = max(0, r * 128)
                            w = slice(col0, QB)
                            qsw = slice(j * QB + col0, (j + 1) * QB)
                            sps = psS.tile([128, QB], f32,
                                           name=f"sps_{h}_{j}_{c}", tag="s")
                            nc.tensor.matmul(sps[:, w], lhsT=kn[h][:, ks],
                                             rhs=qn[:, qsw], start=True,
                                             stop=False,
                                             skip_group_check=True)
                            nc.tensor.matmul(sps[:, w],
                                             lhsT=kr[off:off + 64, ks],
                                             rhs=qr[off:off + 64, qsw],
                                             start=False, stop=(r < 0),
                                             skip_group_check=True)
                            if r >= 0:
                                # Add the causal tri mask on the PE itself
                                # (identity @ tri) so exp never waits on a
                                # cross-engine DVE hop.
                                nc.tensor.matmul(
                                    sps[:, col0:col0 + 128], lhsT=ident_sb,
                                    rhs=masks_sb, start=False, stop=True,
                                    skip_group_check=True)
                            pt = apool.tile([128, QB], cdt,
                                            name=f"pt_{h}_{j}_{c}", tag="pt",
                                            bufs=4)
                            nc.scalar.activation(pt[:, w], sps[:, w], Exp)
                            nc.tensor.matmul(
                                ops[:, w], lhsT=vt[c][:, h * DV:(h + 1) * DV],
                                rhs=pt[:, w], start=(c == 0),
                                stop=(c == nch - 1), skip_group_check=True)
                            if c == 0:
                                nc.vector.tensor_copy(acc, pt)
                            else:
                                nc.vector.tensor_add(acc[:, w], acc[:, w],
                                                     pt[:, w])
                            if c == 1 and pend_den is not None:
                                emit_den(*pend_den)
                                pend_den = None
                            if c == 3 and pend_norm is not None:
                                norm_late(*pend_norm)
                                pend_norm = None
                        accb = apool.tile([128, QB], cdt,
                                          name=f"accb_{h}_{j}", tag="accb",
                                          bufs=2)
                        nc.vector.tensor_copy(accb, acc)
                        pend_den = (h, j, accb, ops)
                        if h == HP - 1 and j < NQB - 1:
                            # outT[:, block j+1] is normalized for all heads
                            # by now; project it while attention continues.
                            emit_wo_block(j + 1)
                emit_den(*pend_den)
                norm_late(*pend_norm)
                emit_wo_block(0)

                # ---- Output projection; PSUM reuses the score slots ----


    if split_waits:
        split_multi_waits()
    return nc


def get_program(split_waits=True):
    if split_waits not in _PROGRAM:
        _PROGRAM[split_waits] = _build_program(split_waits)
    return _PROGRAM[split_waits]


def make_core_inputs(x, Wq, Wkv_a, Wkv_b, Wo):
    """Host-side sharding/pre-processing. Returns list of 8 input dicts."""
    scale = 1.0 / math.sqrt(DN + DR)

    inv_freq = 1.0 / (ROPE_THETA ** (np.arange(0, DR, 2, dtype=np.float64) / DR))
    t = np.arange(S, dtype=np.float64)
    freqs = np.outer(t, inv_freq)                      # [S, 32]
    cos32 = np.cos(freqs).T.astype(np.float32)         # [32, S]
    sin32 = np.sin(freqs).T.astype(np.float32)
    cosf = np.tile(cos32, (4, 1)).astype(BF16)         # [128, S]
    sinf = np.tile(np.concatenate([-sin32, sin32], axis=0), (2, 1)).astype(BF16)

    row = np.arange(128)[:, None]
    col = np.arange(128)[None, :]
    masks = np.where(col >= row, 0.0, -1e30).astype(BF16)  # [128, 128]
    ident = np.eye(128, dtype=BF16)
    ones = np.ones([128, 1], dtype=BF16)
    onesf = np.ones([1, 128], dtype=BF16)

    Wq_r = np.asarray(Wq, dtype=np.float32).reshape(D_MODEL, N_HEADS, DN + DR)
    Wb_r = np.asarray(Wkv_b, dtype=np.float32).reshape(R, N_HEADS, DN + DV)
    Wo_f = np.asarray(Wo, dtype=np.float32)
    Wkva_f = np.asarray(Wkv_a, dtype=np.float32).astype(BF16)
    x_f = np.asarray(x, dtype=np.float32)

    def pair_rows(a):
        """[2K, N] -> [K, 2N] with row c*128+p = [a[256c+p] | a[256c+128+p]]."""
        k2, n = a.shape
        out = np.empty((k2 // 2, 2 * n), dtype=a.dtype)
        for c in range(k2 // 256):
            out[c * 128:(c + 1) * 128, :n] = a[256 * c:256 * c + 128]
            out[c * 128:(c + 1) * 128, n:] = a[256 * c + 128:256 * c + 256]
        return out

    in_maps = []
    x8_cache = {}
    for c in range(NCORES):
        b, g = divmod(c, HP)
        heads = list(range(HP * g, HP * g + HP))
        xTc = np.ascontiguousarray(x_f[b].T)
        if b not in x8_cache:
            x8_cache[b] = np.ascontiguousarray(
                pair_rows((xTc * SX).astype(F8)))
        wq_nope = Wq_r[:, heads, :DN].reshape(D_MODEL, HP * DN)
        wq_rope = Wq_r[:, heads, DN:].reshape(D_MODEL, HP * DR)
        wq_c = np.concatenate([wq_nope, wq_rope], axis=1) * scale
        wq8_c = pair_rows((wq_c * SW).astype(F8))
        wbk8_c = pair_rows(
            (Wb_r[:, heads, :DN].reshape(R, HP * DN) * 512.0).astype(F8))
        wbv_c = np.ascontiguousarray(
            Wb_r[:, heads, DN:].reshape(R, HP * DV)).astype(BF16)
        wo_c = np.ascontiguousarray(
            Wo_f[HP * g * DV:(HP * g + HP) * DV, :]).astype(BF16)
        in_maps.append({
            "xT": xTc.astype(BF16),
            "x8d": x8_cache[b],
            "wq8d": np.ascontiguousarray(wq8_c),
            "wkva": Wkva_f,
            "wkvbk8": np.ascontiguousarray(wbk8_c),
            "wkvbv": wbv_c,
            "wo": wo_c,
            "cosf": cosf,
            "sinf": sinf,
            "masks": masks,
            "ident": ident,
            "ones": ones,
            "onesf": onesf,
        })
    return in_maps


def gather_output(results):
    """results: list of 8 dicts with 'outp' [D_MODEL, S] bf16 partials."""
    out = np.empty((B, S, D_MODEL), dtype=np.float32)
    for b in range(B):
        acc = results[HP * b]["outp"].astype(np.float32)
        for g in range(1, HP):
            acc = acc + results[HP * b + g]["outp"].astype(np.float32)
        out[b] = acc.T
    return out


def kernel(x, Wq, Wkv_a, Wkv_b, Wo):
    from concourse.bass_utils import run_bass_kernel_spmd

    nc = get_program()
    in_maps = make_core_inputs(x, Wq, Wkv_a, Wkv_b, Wo)
    res = run_bass_kernel_spmd(nc, in_maps, list(range(NCORES)))
    return gather_output(res.results)


# revision 26
# speedup vs baseline: 1.1121x; 1.1047x over previous
"""Multi-Head Latent Attention (MLA) TRN2 Bass kernel.

Sharding: data-parallel over batch (B=2) x tensor-parallel over heads
(16 heads -> 4 per core) = 8 cores. The kv_lora latent path and shared
rope key are computed replicated within each batch group; the final
output projection is computed as per-core partials which the host sums.

All on-device dataflow is "transposed" (feature dim on partitions,
sequence on the free dim) so no PE transposes are ever needed:
  qT      = Wq_perm^T @ xT          [768, S]   via fp8-e4m3 DoubleRow
                                               matmuls (K=256/step, 2x rate)
  kv_aT   = Wkv_a^T @ xT            [576, S]   bf16 (c_kvT rows 0..511,
                                               k_ropeT rows 512..575)
  k_nopeT = Wkv_b_k^T @ c_kvT       [512, S]
  v       = (c_kvT chunk)^T-matmuls [S, 512]   (natural layout)
  RoPE applied in transposed layout with a partition-swap DMA + 3 DVE ops
  scoresT[s_k, s_q] per (head, q-block of 512), causal masks added on the
  4 diagonal chunks, exp on ACT (no max subtraction; scores are bounded),
  softmax denominators via DVE accumulation of the exp tiles + a single
  ones-matmul per (head, q-block), out^T accumulated in PSUM, normalized
  by broadcasted reciprocals, then partialT = Wo_c^T @ outT in bf16.

The q-projection runs in fp8: host supplies x and Wq quantized to e4m3
(scales 16 and 4096) in the DoubleRow pair layout; measured end-to-end
error ~1.2e-2 vs the 2e-2 gate.
"""

import math
import sys

import numpy as np
import ml_dtypes

try:  # concourse ships in the container; fall back to the repo checkout
    import concourse.bass  # noqa: F401
except ImportError:  # pragma: no cover
    for p in ("/opt/trn_rl_repo", "/root/.axon_site/_ro/trn_rl_repo"):
        if p not in sys.path:
            sys.path.insert(0, p)

# Problem constants (hardcoded; harness calls kernel() standalone).
D_MODEL = 2048
N_HEADS = 16
R = 512          # kv lora rank
DN = 128         # d_nope
DR = 64          # d_rope
DV = 128         # d_v
ROPE_THETA = 10000.0
B = 2
S = 2048
HP = 4           # heads per core
QB = 512         # q block size
NKC = S // 128   # 16 k chunks
NQB = S // QB    # 4 q blocks
NCORES = 8

BF16 = ml_dtypes.bfloat16
F8 = ml_dtypes.float8_e4m3fn
SX = 16.0        # fp8 scale on x
SW = 4096.0      # fp8 scale on (Wq * softmax_scale)
QSCALE = 1.0 / (SX * SW)

_PROGRAM = {}


def _build_program(split_waits=True):
    import concourse.bass as bass
    import concourse.mybir as mybir
    from concourse.tile import TileContext

    def split_multi_waits(max_waits=1):
        """The walrus build in this container rejects instructions with
        more than `max_waits` sync-wait commands. Move excess waits onto
        same-engine NoOps inserted just before the instruction."""
        for f in nc.m.functions:
            for bb in f.blocks:
                out = []
                changed = False
                for inst in bb.instructions:
                    si = getattr(inst, "sync_info", None)
                    ws = list(si.on_wait) if si is not None else []
                    if len(ws) > max_waits:
                        changed = True
                        inst.sync_info = mybir.SyncInfo(
                            on_wait=ws[:max_waits],
                            on_update=list(si.on_update))
                        for w in ws[max_waits:]:
                            n = mybir.InstNoOp(
                                name=nc.get_next_instruction_name(),
                                ins=[], outs=[])
                            n.engine = inst.engine
                            n.sync_info = mybir.SyncInfo(
                                on_wait=[w], on_update=[])
                            out.append(n)
                    out.append(inst)
                if changed:
                    bb.instructions = out

    f32 = mybir.dt.float32
    cdt = mybir.dt.bfloat16
    f8 = mybir.dt.float8e4
    DRMODE = mybir.MatmulPerfMode.DoubleRow

    nc = bass.Bass()

    xT = nc.dram_tensor("xT", [D_MODEL, S], cdt, kind="ExternalInput")
    # fp8 pair layout: row c*128+p holds [xT[256c+p, :] | xT[256c+128+p, :]]
    x8d = nc.dram_tensor("x8d", [D_MODEL // 2, 2 * S], f8, kind="ExternalInput")
    wq8d = nc.dram_tensor("wq8d", [D_MODEL // 2, 2 * HP * (DN + DR)], f8,
                          kind="ExternalInput")
    wkva = nc.dram_tensor("wkva", [D_MODEL, R + DR], cdt, kind="ExternalInput")
    wkvbk8 = nc.dram_tensor("wkvbk8", [R // 2, 2 * HP * DN], f8,
                            kind="ExternalInput")
    wkvbv = nc.dram_tensor("wkvbv", [R, HP * DV], cdt, kind="ExternalInput")
    wo = nc.dram_tensor("wo", [HP * DV, D_MODEL], cdt, kind="ExternalInput")
    cosf = nc.dram_tensor("cosf", [128, S], cdt, kind="ExternalInput")
    sinf = nc.dram_tensor("sinf", [128, S], cdt, kind="ExternalInput")
    masks = nc.dram_tensor("masks", [128, 128], cdt, kind="ExternalInput")
    ident = nc.dram_tensor("ident", [128, 128], cdt, kind="ExternalInput")
    ones = nc.dram_tensor("ones", [128, 1], cdt, kind="ExternalInput")
    onesf = nc.dram_tensor("onesf", [1, 128], cdt, kind="ExternalInput")
    outp = nc.dram_tensor("outp", [D_MODEL, S], cdt, kind="ExternalOutput")

    Exp = mybir.ActivationFunctionType.Exp
    Ln = mybir.ActivationFunctionType.Ln

    NMT = HP * (DN + DR) // 128  # 6 qT M-tiles

    with TileContext(nc) as tc:
        with (
            tc.tile_pool(name="const", bufs=1) as cpool,
            tc.tile_pool(name="persist", bufs=1) as ppool,
        ):
            cosf_sb = cpool.tile([128, S], cdt, name="cosf_sb")
            sinf_sb = cpool.tile([128, S], cdt, name="sinf_sb")
            masks_sb = cpool.tile([128, 128], cdt, name="masks_sb")
            ident_sb = cpool.tile([128, 128], cdt, name="ident_sb")
            ones_sb = cpool.tile([128, 1], cdt, name="ones_sb")
            onesb_sb = cpool.tile([1, 128], cdt, name="onesb_sb")

            # Persistent activations.
            qT = [
                ppool.tile([128, S], cdt, name=f"qT{m}", tag="qT", bufs=6)
                for m in range(6)
            ]
            ck = [
                ppool.tile([128, S], cdt, name=f"ck{m}", tag="cko", bufs=4)
                for m in range(4)
            ]
            kn = [
                ppool.tile([128, S], cdt, name=f"kn{m}", tag="kn", bufs=4)
                for m in range(4)
            ]
            kr = ppool.tile([128, S], cdt, name="krope", tag="krope", bufs=1)
            vt = [
                ppool.tile([128, HP * DV], cdt, name=f"v{i}", tag="v", bufs=NKC)
                for i in range(NKC)
            ]
            # RoPE swap scratch lives in the persistent pool so the kvT
            # weight pool does not WAR-serialize against the rope phase.
            swt = [
                ppool.tile([128, S], cdt, name=f"sw{i}", tag="sw", bufs=3)
                for i in range(3)
            ]

            # kv_b weights persist so their DMAs can issue at startup.
            wbk8_sb = [
                ppool.tile([128, 2, HP * DN], f8, name=f"wbk8_sb{p}",
                           tag="wbk8", bufs=2)
                for p in range(2)
            ]
            wbv_sb = [
                ppool.tile([128, HP * DV], cdt, name=f"wbv_sb{r}", tag="wbv",
                           bufs=4)
                for r in range(4)
            ]
            # fp8 copy of c_kvT in DoubleRow pair layout for the k_nope
            # up-projection (pair p holds chunks 2p, 2p+1); one contiguous
            # tile per (q-block, pair) so the matmul ifmap stream stays
            # contiguous.
            ck8p = [
                [
                    ppool.tile([128, 2, QB], f8, name=f"ck8p_{t}_{p}",
                               tag="ck8", bufs=8)
                    for p in range(2)
                ]
                for t in range(NQB)
            ]

            # ---- Phase 1: x projections, then kv up-projection ----
            with (
                tc.tile_pool(name="wproj", bufs=1) as wpool,
                tc.tile_pool(name="xstream", bufs=1) as xpool,
                tc.tile_pool(name="psA", bufs=8, space="PSUM") as psA,
            ):
                # Quarter-0 DMAs, interleaved across four queues so the
                # first matmuls can start early: fp8 x pairs on gpsimd,
                # bf16 x chunks on sync, fp8 wq on scalar, wkva on vector.
                wq8_sb = []
                wkva_sb = []
                xq0 = []
                x80 = []
                NM = HP * (DN + DR)
                for c in range(8):
                    w8 = wpool.tile([128, 2, NM], f8,
                                    name=f"wq8_{c}", tag="wq8", bufs=8)
                    src3 = wq8d[c * 128:(c + 1) * 128, :].rearrange(
                        "p (two m) -> p two m", two=2)
                    if c == 0:
                        # Split the first weight load so the fp8 ladder can
                        # start after half the tile lands.
                        nc.scalar.dma_start(w8[:, :, 0:NM // 2],
                                            src3[:, :, 0:NM // 2])
                        nc.scalar.dma_start(w8[:, :, NM // 2:NM],
                                            src3[:, :, NM // 2:NM])
                    else:
                        nc.scalar.dma_start(w8, src3)
                    wq8_sb.append(w8)
                    x8t = xpool.tile([128, 2, QB], f8, name=f"x8_0_{c}",
                                     tag="x8", bufs=12)
                    src = x8d[c * 128:(c + 1) * 128, :].rearrange(
                        "p (two s) -> p two s", two=2)[:, :, 0:QB]
                    nc.scalar.dma_start(x8t, src)
                    x80.append(x8t)
                    for k in (2 * c, 2 * c + 1):
                        xk = xpool.tile([128, QB], cdt, name=f"xq_0_{k}",
                                        tag="xq0", bufs=16)
                        nc.sync.dma_start(xk, xT[k * 128:(k + 1) * 128, 0:QB])
                        xq0.append(xk)
                        w2 = wpool.tile([128, R + DR], cdt,
                                        name=f"wkva_sb{k}", tag="wkva",
                                        bufs=16)
                        nc.gpsimd.dma_start(w2, wkva[k * 128:(k + 1) * 128, :])
                        wkva_sb.append(w2)
                for p in range(2):
                    nc.sync.dma_start(
                        wbk8_sb[p].rearrange("p two m -> p (two m)"),
                        wkvbk8[p * 128:(p + 1) * 128, :])
                for r in range(4):
                    nc.sync.dma_start(wbv_sb[r], wkvbv[r * 128:(r + 1) * 128, :])
                nc.gpsimd.dma_start(cosf_sb, cosf[:, :])
                nc.gpsimd.dma_start(sinf_sb, sinf[:, :])
                nc.gpsimd.dma_start(masks_sb, masks[:, :])
                nc.gpsimd.dma_start(ident_sb, ident[:, :])
                nc.gpsimd.dma_start(ones_sb, ones[:, :])
                nc.gpsimd.dma_start(onesb_sb, onesf[:, :])

                # Quarter 0 is DMA-latency bound: run the contraction OUTER
                # over 8 PSUM banks (6 qT fp8 ladders + ck0/ck1 bf16) so
                # each arriving x chunk gets work immediately.
                t0 = slice(0, QB)
                ps8 = [
                    psA.tile([128, QB], f32, name=f"psq0_{m}", tag="ps")
                    for m in range(8)
                ]
                for s in range(16):
                    for m in range(2):
                        nc.tensor.matmul(
                            ps8[6 + m], lhsT=wkva_sb[s][:, m * 128:(m + 1) * 128],
                            rhs=xq0[s], start=(s == 0), stop=(s == 15))
                    if s % 2 == 1:
                        c = s // 2
                        for m in range(NMT):
                            nc.tensor.matmul(
                                ps8[m],
                                lhsT=wq8_sb[c][:, :, m * 128:(m + 1) * 128],
                                rhs=x80[c], start=(c == 0), stop=(c == 7),
                                perf_mode=DRMODE)
                for m in range(6):
                    nc.scalar.mul(qT[m][:, t0], ps8[m], QSCALE)
                for m in range(2):
                    nc.vector.tensor_copy(ck[m][:, t0], ps8[6 + m])
                    nc.vector.tensor_scalar_mul(
                        ck8p[0][m // 2][:, m % 2, :], ck[m][:, t0], SX)
                for m in (2, 3):
                    ps = psA.tile([128, QB], f32, name=f"psk_0_{m}", tag="ps")
                    for k in range(16):
                        nc.tensor.matmul(
                            ps, lhsT=wkva_sb[k][:, m * 128:(m + 1) * 128],
                            rhs=xq0[k], start=(k == 0), stop=(k == 15))
                    nc.vector.tensor_copy(ck[m][:, t0], ps)
                    nc.vector.tensor_scalar_mul(
                        ck8p[0][m // 2][:, m % 2, :], ck[m][:, t0], SX)
                ps = psA.tile([64, QB], f32, name="psr_0", tag="ps")
                for k in range(16):
                    nc.tensor.matmul(
                        ps, lhsT=wkva_sb[k][:, R:R + DR],
                        rhs=xq0[k], start=(k == 0), stop=(k == 15))
                nc.scalar.copy(kr[0:64, t0], ps)
                nc.scalar.copy(kr[64:128, t0], ps)

                for t in range(1, NQB):
                    tcols = slice(t * QB, (t + 1) * QB)
                    # x DMAs for this block: 4 batched bf16 + 8 fp8 pairs.
                    xqb = []
                    for g in range(4):
                        xb = xpool.tile([128, 4, QB], cdt, name=f"xqb_{t}_{g}",
                                        tag="xqb", bufs=6)
                        src = xT[:, tcols].rearrange(
                            "(c p) s -> p c s", p=128)[:, 4 * g:4 * g + 4, :]
                        nc.sync.dma_start(xb, src)
                        xqb.append(xb)
                    x8b = []
                    for c in range(8):
                        x8t = xpool.tile([128, 2, QB], f8, name=f"x8_{t}_{c}",
                                         tag="x8", bufs=12)
                        src = x8d[c * 128:(c + 1) * 128, :].rearrange(
                            "p (two s) -> p two s", two=2)[:, :, tcols]
                        nc.gpsimd.dma_start(x8t, src)
                        x8b.append(x8t)

                    def xqc(k):
                        return xqb[k // 4][:, k % 4, :]

                    # qT M-tiles (fp8 DoubleRow, 8 contraction steps)
                    for m in range(NMT):
                        ps = psA.tile([128, QB], f32, name=f"psq_{t}_{m}",
                                      tag="ps")
                        for c in range(8):
                            nc.tensor.matmul(
                                ps, lhsT=wq8_sb[c][:, :, m * 128:(m + 1) * 128],
                                rhs=x8b[c], start=(c == 0), stop=(c == 7),
                                perf_mode=DRMODE)
                        nc.scalar.mul(qT[m][:, tcols], ps, QSCALE)
                    # c_kvT M-tiles (bf16)
                    for m in range(4):
                        ps = psA.tile([128, QB], f32, name=f"psk_{t}_{m}",
                                      tag="ps")
                        for k in range(16):
                            nc.tensor.matmul(
                                ps, lhsT=wkva_sb[k][:, m * 128:(m + 1) * 128],
                                rhs=xqc(k), start=(k == 0), stop=(k == 15))
                        nc.vector.tensor_copy(ck[m][:, tcols], ps)
                        nc.vector.tensor_scalar_mul(
                            ck8p[t][m // 2][:, m % 2, :], ck[m][:, tcols], SX)
                    # k_ropeT (rows 512..575 of kv_aT), duplicated into kr
                    ps = psA.tile([64, QB], f32, name=f"psr_{t}", tag="ps")
                    for k in range(16):
                        nc.tensor.matmul(
                            ps, lhsT=wkva_sb[k][:, R:R + DR],
                            rhs=xqc(k), start=(k == 0), stop=(k == 15))
                    nc.scalar.copy(kr[0:64, tcols], ps)
                    nc.scalar.copy(kr[64:128, tcols], ps)

                # ---- RoPE rotation (in place; DVE work overlaps the
                # kv up-projection matmuls below) ----
                # rot = x * cosf + swap32(x) * sinf, where swap32 swaps each
                # 32-row half within every 64-row group (signs in sinf).
                for idx, tap in enumerate([qT[4], qT[5], kr]):
                    sw = swt[idx]
                    for blk in range(4):
                        src = (blk ^ 1) * 32
                        nc.sync.dma_start(
                            sw[blk * 32:(blk + 1) * 32, :],
                            tap[src:src + 32, :])
                    nc.vector.tensor_mul(tap, tap, cosf_sb)
                    nc.vector.tensor_mul(sw, sw, sinf_sb)
                    nc.vector.tensor_add(tap, tap, sw)

                # ---- kv up-projection (k_nopeT, v); same PSUM pool ----
                # k_nope runs in fp8 DoubleRow (K=256 per step).
                for m in range(4):
                    for nb in range(NQB):
                        ncols = slice(nb * QB, (nb + 1) * QB)
                        ps = psA.tile([128, QB], f32, name=f"psn_{m}_{nb}",
                                      tag="ps")
                        for p in range(2):
                            nc.tensor.matmul(
                                ps, lhsT=wbk8_sb[p][:, :, m * 128:(m + 1) * 128],
                                rhs=ck8p[nb][p], start=(p == 0),
                                stop=(p == 1), perf_mode=DRMODE)
                        nc.scalar.mul(kn[m][:, ncols], ps, 1.0 / (SX * 512.0))
                for i in range(NKC):
                    ps = psA.tile([128, HP * DV], f32, name=f"psv_{i}", tag="ps")
                    for r in range(4):
                        nc.tensor.matmul(
                            ps, lhsT=ck[r][:, i * 128:(i + 1) * 128],
                            rhs=wbv_sb[r], start=(r == 0), stop=(r == 3))
                    nc.vector.tensor_copy(vt[i], ps)

            # outT tiles reuse the c_kvT slots (same tag, 4 bufs).
            outT = [
                ppool.tile([128, S], cdt, name=f"outT{h}", tag="cko", bufs=4)
                for h in range(HP)
            ]

            # ---- Phase 3: attention + output projection ----
            with (
                tc.tile_pool(name="att", bufs=1) as apool,
                tc.tile_pool(name="psS", bufs=4, space="PSUM") as psS,
                tc.tile_pool(name="psO", bufs=2, space="PSUM") as psO,
                tc.tile_pool(name="psD", bufs=1, space="PSUM") as psD,
                tc.tile_pool(name="psBC", bufs=1, space="PSUM") as psBC,
            ):
                # Wo loads overlap the attention phase on the idle sync queue.
                wo_sb = [
                    apool.tile([128, D_MODEL], cdt, name=f"wo_sb{r}", tag="wo",
                               bufs=4)
                    for r in range(4)
                ]
                for r in range(4):
                    nc.sync.dma_start(wo_sb[r], wo[r * 128:(r + 1) * 128, :])

                pend_den = None   # (h, j, accb, ops)
                pend_norm = None  # (h, j, ops, recb)

                def emit_den(h, j, accb, ops):
                    # Single ones-matmul over the DVE-accumulated exp sums,
                    # then 1/denom as exp(-ln(d)) on the ACT engine.
                    nonlocal pend_norm
                    dps = psD.tile([1, QB], f32, name=f"dps_{h}_{j}", tag="d")
                    nc.tensor.matmul(dps, lhsT=ones_sb, rhs=accb,
                                     start=True, stop=True)
                    rec = apool.tile([1, QB], f32, name=f"rec_{h}_{j}",
                                     tag="rec", bufs=2)
                    nc.scalar.activation(rec, dps, Ln)
                    recb = apool.tile([1, QB], cdt, name=f"recb_{h}_{j}",
                                      tag="recb", bufs=2)
                    nc.scalar.activation(recb, rec, Exp, scale=-1.0)
                    pend_norm = (h, j, ops, recb)

                def norm_late(h, j, ops, recb):
                    # Broadcast 1/denom across partitions via a K=1 matmul,
                    # then scale the out accumulator into outT.
                    qs = slice(j * QB, (j + 1) * QB)
                    bps = psBC.tile([128, QB], f32, name=f"bps_{h}_{j}",
                                    tag="b")
                    nc.tensor.matmul(bps, lhsT=onesb_sb, rhs=recb,
                                     start=True, stop=True)
                    bc = apool.tile([128, QB], f32, name=f"bc_{h}_{j}",
                                    tag="bc", bufs=2)
                    nc.scalar.copy(bc, bps)
                    nc.vector.tensor_mul(outT[h][:, qs], ops, bc)

                def emit_wo_block(nb):
                    # One 512-column block of the output projection; all 16
                    # M-tiles. Emitted as soon as every head's outT for this
                    # block is normalized, so the projection overlaps the
                    # tail of the attention phase.
                    ncols = slice(nb * QB, (nb + 1) * QB)
                    for m in range(16):
                        wopool, wotag = (psS, "s") if m % 2 == 0 else (psO, "o")
                        ps = wopool.tile([128, QB], f32, name=f"psw_{m}_{nb}",
                                         tag=wotag)
                        for r in range(4):
                            nc.tensor.matmul(
                                ps, lhsT=wo_sb[r][:, m * 128:(m + 1) * 128],
                                rhs=outT[r][:, ncols], start=(r == 0),
                                stop=(r == 3))
                        st = apool.tile([128, QB], cdt, name=f"st_{m}_{nb}",
                                        tag="st", bufs=4)
                        if m % 2 == 0:
                            nc.scalar.copy(st, ps)
                        else:
                            nc.vector.tensor_copy(st, ps)
                        dma_eng = (nc.sync, nc.scalar, nc.gpsimd)[m % 3]
                        dma_eng.dma_start(
                            outp[m * 128:(m + 1) * 128, ncols], st)

                # Descending j per head: every normalize chain (DVE sums ->
                # ones-matmul -> Ln/Exp -> bcast) then hides inside a large
                # (12-16 chunk) follower group instead of a 4-chunk one.
                for h in range(HP):
                    qn = qT[h]
                    qr = qT[4 + h // 2]
                    off = (h % 2) * 64
                    for j in range(NQB - 1, -1, -1):
                        qs = slice(j * QB, (j + 1) * QB)
                        ops = psO.tile([128, QB], f32, name=f"ops_{h}_{j}",
                                       tag="o")
                        acc = apool.tile([128, QB], f32, name=f"acc_{h}_{j}",
                                         tag="acc", bufs=2)
                        nch = 4 * (j + 1)
                        for c in range(nch):
                            ks = slice(c * 128, (c + 1) * 128)
                            r = c - 4 * j
                            # Diagonal chunks only need columns >= r*128
                            # (everything to the left is strictly above the
                            # causal boundary). Chunk 0 always start-covers
                            # the full accumulator width.
                            col0 = max(0, r * 128)
                            w = slice(col0, QB)
                            qsw = slice(j * QB + col0, (j + 1) * QB)
                            sps = psS.tile([128, QB], f32,
                                           name=f"sps_{h}_{j}_{c}", tag="s")
                            nc.tensor.matmul(sps[:, w], lhsT=kn[h][:, ks],
                                             rhs=qn[:, qsw], start=True,
                                             stop=False,
                                             skip_group_check=True)
                            nc.tensor.matmul(sps[:, w],
                                             lhsT=kr[off:off + 64, ks],
                                             rhs=qr[off:off + 64, qsw],
                                             start=False, stop=(r < 0),
                                             skip_group_check=True)
                            if r >= 0:
                                # Add the causal tri mask on the PE itself
                                # (identity @ tri) so exp never waits on a
                                # cross-engine DVE hop.
                                nc.tensor.matmul(
                                    sps[:, col0:col0 + 128], lhsT=ident_sb,
                                    rhs=masks_sb, start=False, stop=True,
                                    skip_group_check=True)
                            pt = apool.tile([128, QB], cdt,
                                            name=f"pt_{h}_{j}_{c}", tag="pt",
                                            bufs=4)
                            nc.scalar.activation(pt[:, w], sps[:, w], Exp)
                            nc.tensor.matmul(
                                ops[:, w], lhsT=vt[c][:, h * DV:(h + 1) * DV],
                                rhs=pt[:, w], start=(c == 0),
                                stop=(c == nch - 1), skip_group_check=True)
                            if c == 0:
                                nc.vector.tensor_copy(acc, pt)
                            else:
                                nc.vector.tensor_add(acc[:, w], acc[:, w],
                                                     pt[:, w])
                            if c == 1 and pend_den is not None:
                                emit_den(*pend_den)
                                pend_den = None
                            if c == 3 and pend_norm is not None:
                                norm_late(*pend_norm)
                                pend_norm = None
                        accb = apool.tile([128, QB], cdt,
                                          name=f"accb_{h}_{j}", tag="accb",
                                          bufs=2)
                        nc.vector.tensor_copy(accb, acc)
                        pend_den = (h, j, accb, ops)
                        if h == HP - 1 and j < NQB - 1:
                            # outT[:, block j+1] is normalized for all heads
                            # by now; project it while attention continues.
                            emit_wo_block(j + 1)
                emit_den(*pend_den)
                norm_late(*pend_norm)
                emit_wo_block(0)

                # ---- Output projection; PSUM reuses the score slots ----


    if split_waits:
        split_multi_waits()
    return nc


def get_program(split_waits=True):
    if split_waits not in _PROGRAM:
        _PROGRAM[split_waits] = _build_program(split_waits)
    return _PROGRAM[split_waits]


def make_core_inputs(x, Wq, Wkv_a, Wkv_b, Wo):
    """Host-side sharding/pre-processing. Returns list of 8 input dicts."""
    scale = 1.0 / math.sqrt(DN + DR)

    inv_freq = 1.0 / (ROPE_THETA ** (np.arange(0, DR, 2, dtype=np.float64) / DR))
    t = np.arange(S, dtype=np.float64)
    freqs = np.outer(t, inv_freq)                      # [S, 32]
    cos32 = np.cos(freqs).T.astype(np.float32)         # [32, S]
    sin32 = np.sin(freqs).T.astype(np.float32)
    cosf = np.tile(cos32, (4, 1)).astype(BF16)         # [128, S]
    sinf = np.tile(np.concatenate([-sin32, sin32], axis=0), (2, 1)).astype(BF16)

    row = np.arange(128)[:, None]
    col = np.arange(128)[None, :]
    masks = np.where(col >= row, 0.0, -1e30).astype(BF16)  # [128, 128]
    ident = np.eye(128, dtype=BF16)
    ones = np.ones([128, 1], dtype=BF16)
    onesf = np.ones([1, 128], dtype=BF16)

    Wq_r = np.asarray(Wq, dtype=np.float32).reshape(D_MODEL, N_HEADS, DN + DR)
    Wb_r = np.asarray(Wkv_b, dtype=np.float32).reshape(R, N_HEADS, DN + DV)
    Wo_f = np.asarray(Wo, dtype=np.float32)
    Wkva_f = np.asarray(Wkv_a, dtype=np.float32).astype(BF16)
    x_f = np.asarray(x, dtype=np.float32)

    def pair_rows(a):
        """[2K, N] -> [K, 2N] with row c*128+p = [a[256c+p] | a[256c+128+p]]."""
        k2, n = a.shape
        out = np.empty((k2 // 2, 2 * n), dtype=a.dtype)
        for c in range(k2 // 256):
            out[c * 128:(c + 1) * 128, :n] = a[256 * c:256 * c + 128]
            out[c * 128:(c + 1) * 128, n:] = a[256 * c + 128:256 * c + 256]
        return out

    in_maps = []
    x8_cache = {}
    for c in range(NCORES):
        b, g = divmod(c, HP)
        heads = list(range(HP * g, HP * g + HP))
        xTc = np.ascontiguousarray(x_f[b].T)
        if b not in x8_cache:
            x8_cache[b] = np.ascontiguousarray(
                pair_rows((xTc * SX).astype(F8)))
        wq_nope = Wq_r[:, heads, :DN].reshape(D_MODEL, HP * DN)
        wq_rope = Wq_r[:, heads, DN:].reshape(D_MODEL, HP * DR)
        wq_c = np.concatenate([wq_nope, wq_rope], axis=1) * scale
        wq8_c = pair_rows((wq_c * SW).astype(F8))
        wbk8_c = pair_rows(
            (Wb_r[:, heads, :DN].reshape(R, HP * DN) * 512.0).astype(F8))
        wbv_c = np.ascontiguousarray(
            Wb_r[:, heads, DN:].reshape(R, HP * DV)).astype(BF16)
        wo_c = np.ascontiguousarray(
            Wo_f[HP * g * DV:(HP * g + HP) * DV, :]).astype(BF16)
        in_maps.append({
            "xT": xTc.astype(BF16),
            "x8d": x8_cache[b],
            "wq8d": np.ascontiguousarray(wq8_c),
            "wkva": Wkva_f,
            "wkvbk8": np.ascontiguousarray(wbk8_c),
            "wkvbv": wbv_c,
            "wo": wo_c,
            "cosf": cosf,
            "sinf": sinf,
            "masks": masks,
            "ident": ident,
            "ones": ones,
            "onesf": onesf,
        })
    return in_maps


def gather_output(results):
    """results: list of 8 dicts with 'outp' [D_MODEL, S] bf16 partials."""
    out = np.empty((B, S, D_MODEL), dtype=np.float32)
    for b in range(B):
        acc = results[HP * b]["outp"].astype(np.float32)
        for g in range(1, HP):
            acc = acc + results[HP * b + g]["outp"].astype(np.float32)
        out[b] = acc.T
    return out


def kernel(x, Wq, Wkv_a, Wkv_b, Wo):
    from concourse.bass_utils import run_bass_kernel_spmd

    nc = get_program()
    in_maps = make_core_inputs(x, Wq, Wkv_a, Wkv_b, Wo)
    res = run_bass_kernel_spmd(nc, in_maps, list(range(NCORES)))
    return gather_output(res.results)
